# revision 47
# baseline (speedup 1.0000x reference)
"""BitNet ternary linear layer on 8 Trainium2 NeuronCores.

out[b, o] = (sum_i w[o,i] * round_clip(x[b,i]/act_scale)) * weight_scale * act_scale + bias[o]
  with w = unpack2bit(packed_weight) - 1   (codes c in {0..3} -> w in {-1..2})
  and  act_scale = max(absmax(x), 1e-5) / 127.

v2 strategy (tensor-parallel, column sharded over out_features):
 - The int32 packed_weight words only carry 8 payload bits (4x 2-bit codes,
   values <= 170). Host casts to uint8 losslessly and transposes to [I/4, OS]
   per core -> device HBM traffic drops 4x (29.4MB -> 7.34MB per core,
   ~20.4us at the ~360GB/s per-core DMA roofline).
 - Device (per core, identical program):
   * quantize x on-chip: absmax -> r=127/absmax -> x_q = rne(x*r) in bf16.
   * stream uint8 weight j-tiles; unpack 2-bit planes on DVE in BLK-jtile
     blocks: one fused tensor_scalar (word >> 2k) & 0x03030303 per plane,
     FD=BLK*896 int32 (write-port bound: 8B/cycle/lane).
   * planes feed the PE as fp8e4 DENORMALS (bytes {0..3} = c*2^-9 exact)
     against the bf16 stationary x_q, col-tiled 4-wide (tile_position).
   * acc in PSUM f32 (exact); epilogue merged over [128,1024]:
     out = acc*512*gamma - gamma*Sx + bias, gamma = weight_scale*act_scale.
"""

import os
import sys

sys.path.insert(0, "/opt/trn_rl_repo")

import numpy as np

import concourse.bacc as bacc
import concourse.mybir as mybir
from concourse import bass_isa
from concourse import tile
from concourse.bass_utils import run_bass_kernel_spmd

AluOp = mybir.AluOpType
dt = mybir.dt

O, I, B = 28672, 8192, 8
NCORES = 8
OS = O // NCORES          # 3584 out-features per core
JB = I // 4               # 2048 packed bytes per out-feature
NJT = JB // 128           # 16 j-tiles
MAGIC = 12582912.0        # 1.5 * 2^23: magic RNE round-to-integer constant

CH = 448                  # o-chunk size: 8 chunks, (g, cc) -> m = 2g+cc
NG = 4                    # PE column groups

_cache = {}
LAST_RESULTS = None       # test harness can inspect run results here


def _build(repeat=1, mode="full", blk=4, ep="dve", acts="xq,sxg,ep",
           dualq=False):
    acts = set(acts.split(",")) if acts else set()
    # mode: "full" = real kernel
    #       "dma"  = weight stream only          (DMA rate)
    #       "dmaplanes" = stream + DVE unpack    (max(DMA, DVE))
    #       "planes" = unpack from resident tile (DVE rate)
    #       "pe"   = matmuls from resident plane (PE rate)
    #       "mm"   = stream + matmuls, no unpack (max(DMA, PE))
    nc = bacc.Bacc("TRN2", target_bir_lowering=False, debug=False)
    NBLK = NJT // blk
    WT = blk * OS             # block tile bytes per partition

    wb = nc.dram_tensor("wb", [JB, OS], dt.uint8, kind="ExternalInput")
    xs = nc.dram_tensor("xs", [128, 512], dt.float32, kind="ExternalInput")
    biasr = nc.dram_tensor("biasr", [8, OS], dt.float32, kind="ExternalInput")
    ws = nc.dram_tensor("ws", [1, 1], dt.float32, kind="ExternalInput")
    out = nc.dram_tensor("out", [8, OS], dt.float32, kind="ExternalOutput")
    AF = mybir.ActivationFunctionType

    with tile.TileContext(nc) as tc:
        with (
            tc.tile_pool(name="io", bufs=2) as io,
            tc.tile_pool(name="wpool", bufs=3 if blk <= 4 else 2) as wpool,
            tc.tile_pool(name="plpool", bufs=3 if blk <= 4 else 2) as plpool,
            tc.tile_pool(name="opool", bufs=2) as opool,
            tc.tile_pool(name="ps", bufs=2, space="PSUM") as ps,
        ):
            xs_t = io.tile([128, 512], dt.float32)
            nc.gpsimd.dma_start(xs_t[:], xs[:])
            ws_t = io.tile([1, 1], dt.float32)
            nc.gpsimd.dma_start(ws_t[:], ws[:])
            magic_t = io.tile([128, 1], dt.float32)
            nc.vector.memset(magic_t[:], MAGIC)
            biasr_t = io.tile([8, OS], dt.float32)
            nc.gpsimd.dma_start(biasr_t[:], biasr[:])
            if ep in ("dve", "psum"):
                biasc = nc.dram_tensor("biasc", [128, 1024], dt.float32,
                                       kind="ExternalInput")
                biasc_t = io.tile([128, 1024], dt.float32)
                nc.gpsimd.dma_start(biasc_t[:], biasc[:])

            if mode in ("planes", "pe"):
                wres = io.tile([128, WT], dt.uint8)
                for t in range(blk):
                    nc.sync.dma_start(wres[:, t * OS:(t + 1) * OS],
                                      wb[t * 128:(t + 1) * 128, :])
            if mode in ("pe", "mm"):
                gbuf = io.tile([128, OS], dt.uint8)
                nc.vector.memset(gbuf[:], 0.0)
                xqg = io.tile([128, 8], dt.bfloat16)
                nc.vector.memset(xqg[:], 1.0)
            if ep == "psum":
                zmm = io.tile([128, 1024], dt.uint8)   # zero fp8 operands for
                nc.vector.memset(zmm[:], 0.0)          # the has_written dummy
                zmm8 = zmm[:].bitcast(dt.float8e4)
            simsafe = os.environ.get("BITNET_SIMSAFE") == "1"
            if mode != "full":
                zt = io.tile([8, OS], dt.float32)
                nc.vector.memset(zt[:], 0.0)

            def body():
                if mode in ("full", "quant"):
                    # ---------- x quantization ----------
                    am_p = io.tile([128, 1], dt.float32)
                    nc.vector.tensor_reduce(
                        am_p[:], xs_t[:], axis=mybir.AxisListType.X, op=AluOp.max,
                        apply_absolute_value=True,
                    )
                    am = io.tile([128, 1], dt.float32)
                    nc.gpsimd.partition_all_reduce(
                        am[:], am_p[:], channels=128,
                        reduce_op=bass_isa.ReduceOp.absmax,
                    )
                    nc.vector.tensor_scalar_max(am[:], am[:], 1e-5)

                    r = io.tile([128, 1], dt.float32)
                    nc.vector.reciprocal(r[:], am[:])
                    nc.vector.tensor_scalar_mul(r[:], r[:], 127.0)

                    ws_b = io.tile([128, 1], dt.float32)
                    nc.gpsimd.partition_broadcast(ws_b[:], ws_t[:])
                    gamma = io.tile([128, 1], dt.float32)
                    nc.vector.tensor_scalar(
                        out=gamma[:], in0=am[:], scalar1=1.0 / 127.0,
                        scalar2=ws_b[:], op0=AluOp.mult, op1=AluOp.mult,
                    )
                    g512 = io.tile([128, 1], dt.float32)
                    nc.vector.tensor_scalar_mul(g512[:], gamma[:], 512.0)

                    # x_q = rne(x*r) via magic rounding, on the (otherwise
                    # idle) scalar engine: per-partition AP scale/bias
                    xq_f = io.tile([128, 512], dt.float32)
                    xq = io.tile([128, 512], dt.bfloat16)
                    if "xq" in acts:
                        nc.scalar.activation(
                            xq_f[:], xs_t[:], AF.Identity,
                            bias=magic_t[:, 0:1], scale=r[:, 0:1],
                        )
                        nc.scalar.activation(
                            xq[:], xq_f[:], AF.Copy, bias=-MAGIC, scale=1.0,
                        )
                    else:
                        nc.vector.tensor_scalar(
                            out=xq_f[:], in0=xs_t[:], scalar1=r[:],
                            scalar2=MAGIC, op0=AluOp.mult, op1=AluOp.add,
                        )
                        nc.vector.tensor_scalar(
                            out=xq[:], in0=xq_f[:], scalar1=MAGIC,
                            scalar2=None, op0=AluOp.subtract,
                        )

                    # Sx*gamma rank-1 correction (codes = w+1)
                    t_pb = io.tile([128, 8], dt.float32)
                    nc.vector.tensor_reduce(
                        t_pb[:],
                        xq[:].rearrange("p (jt k b) -> p b (jt k)",
                                        jt=NJT, k=4, b=8),
                        axis=mybir.AxisListType.X, op=AluOp.add,
                    )
                    sxg_ps = ps.tile([128, 1], dt.float32, tag="sxg")
                    nsxg = io.tile([128, 1], dt.float32)
                    if simsafe:
                        nc.vector.memset(nsxg[:], 0.0)
                    for g in range(NG):
                        nc.tensor.matmul(
                            sxg_ps[32 * g:32 * g + 8, :], t_pb[:], gamma[:],
                            start=True, stop=True, tile_position=(0, 32 * g),
                        )
                        if "sxg" in acts:
                            nc.scalar.mul(
                                nsxg[32 * g:32 * g + 8, :],
                                sxg_ps[32 * g:32 * g + 8, :], -1.0,
                            )
                        else:
                            nc.vector.tensor_scalar_mul(
                                nsxg[32 * g:32 * g + 8, :],
                                sxg_ps[32 * g:32 * g + 8, :], -1.0,
                            )

                if mode == "quant":
                    return

                # ---------- main loop ----------
                if mode == "full" and ep == "acc":
                    # bias lands in out first; the epilogue accum-DMAs add
                    # the scaled acc on the same (FIFO) gpsimd queue
                    nc.gpsimd.dma_start(out[:, :], biasr_t[:])
                if mode in ("full", "pe", "mm"):
                    acc = ps.tile([128, 1024], dt.float32, tag="acc")
                    if simsafe:
                        # sim-only: define the never-written PSUM rows so the
                        # [128,1024] epilogue read passes CoreSim's checker
                        nc.scalar.memzero(acc[:])
                if mode == "full" and ep == "psum":
                    # Pre-load bias/(512*gamma) into PSUM so the matmuls
                    # accumulate straight onto it: a zero matmul with
                    # start=True sets has_written over the whole region, ACT
                    # then overwrites the (zero) values with the scaled bias,
                    # and every real matmul runs start=False. Epilogue needs
                    # no DVE work at all.
                    rg = io.tile([128, 1], dt.float32)
                    nc.vector.reciprocal(rg[:], g512[:])
                    for h in range(2):
                        nc.tensor.matmul(
                            acc[:, h * 512:(h + 1) * 512],
                            zmm8[:, 0:128], zmm8[:, 0:512],
                            start=True, stop=True,
                        )
                    nc.scalar.activation(
                        acc[:], biasc_t[:], AF.Copy, bias=0.0,
                        scale=rg[:, 0:1],
                    )
                for b_ in range(NBLK):
                    if mode in ("full", "dma", "dmaplanes", "mm"):
                        wt = wpool.tile([128, WT], dt.uint8, tag="wt")
                        for t in range(blk):
                            jt = b_ * blk + t
                            eng = (nc.scalar if (dualq and t % 2 == 1)
                                   else nc.sync)
                            eng.dma_start(
                                wt[:, t * OS:(t + 1) * OS],
                                wb[jt * 128:(jt + 1) * 128, :],
                            )
                    if mode in ("dma",):
                        continue
                    src = wres if mode == "planes" else (
                        wt if mode in ("full", "dmaplanes") else None)
                    for k in range(4):
                        if mode in ("full", "dmaplanes", "planes"):
                            pk = plpool.tile([128, WT // 4], dt.int32, tag="pk")
                            if k == 0:
                                nc.vector.tensor_scalar(
                                    out=pk[:], in0=src[:].bitcast(dt.int32),
                                    scalar1=0x03030303, scalar2=None,
                                    op0=AluOp.bitwise_and,
                                )
                            else:
                                nc.vector.tensor_scalar(
                                    out=pk[:], in0=src[:].bitcast(dt.int32),
                                    scalar1=2 * k, scalar2=0x03030303,
                                    op0=AluOp.logical_shift_right,
                                    op1=AluOp.bitwise_and,
                                )
                            pk8 = pk[:].bitcast(dt.float8e4)  # [128, WT]
                        if mode in ("full", "pe", "mm"):
                            for t in range(blk):
                                jt = b_ * blk + t
                                if mode == "full":
                                    lhsT = xq[:, (jt * 4 + k) * 8:
                                              (jt * 4 + k + 1) * 8]
                                else:
                                    lhsT = xqg[:]
                                first = (b_ == 0 and k == 0 and t == 0)
                                last = (b_ == NBLK - 1 and k == 3
                                        and t == blk - 1)
                                for cc in range(2):
                                    for g in range(NG):
                                        m = 2 * g + cc
                                        if mode == "full":
                                            rhs = pk8[:, t * OS + m * CH:
                                                      t * OS + (m + 1) * CH]
                                        else:
                                            rhs = gbuf[:].bitcast(dt.float8e4)[
                                                :, m * CH:(m + 1) * CH]
                                        nc.tensor.matmul(
                                            acc[32 * g:32 * g + 8,
                                                cc * 512:cc * 512 + CH],
                                            lhsT, rhs,
                                            start=(first and ep != "psum"),
                                            stop=last,
                                            skip_group_check=(ep == "psum"),
                                            tile_position=(0, 32 * g),
                                        )

                # ---------- epilogue ----------
                if mode == "full":
                    ot = opool.tile([128, 1024], dt.float32, tag="ot")
                    if "ep" in acts:
                        nc.scalar.activation(
                            ot[:], acc[:], AF.Identity,
                            bias=nsxg[:, 0:1], scale=g512[:, 0:1],
                        )
                    else:
                        nc.vector.tensor_scalar(
                            out=ot[:], in0=acc[:], scalar1=g512[:],
                            scalar2=nsxg[:], op0=AluOp.mult, op1=AluOp.add,
                        )
                    if ep == "dve":
                        nc.vector.tensor_tensor(
                            out=ot[:], in0=ot[:], in1=biasc_t[:], op=AluOp.add,
                        )
                    # ep == "psum": bias already accumulated in PSUM
                    for cc in range(2):
                        for g in range(NG):
                            m = 2 * g + cc
                            if ep == "acc":
                                nc.gpsimd.dma_start(
                                    out[:, m * CH:(m + 1) * CH],
                                    ot[32 * g:32 * g + 8,
                                       cc * 512:cc * 512 + CH],
                                    accum_op=AluOp.add,
                                )
                            else:
                                # ACT's HWDGE queue: keeps the 8 output
                                # stores off the weight-streaming SP queue
                                nc.scalar.dma_start(
                                    out[:, m * CH:(m + 1) * CH],
                                    ot[32 * g:32 * g + 8,
                                       cc * 512:cc * 512 + CH],
                                )

            # repeat==1: plain body (the graded path). repeat>1: perf builds
            # wrap U unrolled bodies in a hardware loop so huge on-device
            # repeat counts compile fast and time accurately (~2us back-edge
            # amortized over U reps).
            if repeat <= 8:
                for _u in range(repeat):
                    body()
            else:
                U = 8
                assert repeat % U == 0, "perf repeat must be a multiple of 8"
                with tc.For_i(0, repeat // U, 1,
                              hint_engines=(mybir.EngineType.PE,)):
                    for _u in range(U):
                        body()

            if mode != "full":
                nc.sync.dma_start(out[:, :], zt[:])

    nc.compile()
    return nc


def prepare_in_maps(inputs):
    """Host-side layout prep shared by kernel() and the perf harness."""
    x = np.asarray(inputs["x"], dtype=np.float32)
    packed_weight = np.asarray(inputs["packed_weight"], dtype=np.int32)
    weight_scale = np.asarray(inputs["weight_scale"], dtype=np.float32)
    bias = np.asarray(inputs["bias"], dtype=np.float32)

    # x -> stationary layout [p, (jt k b)]
    xs_np = np.ascontiguousarray(
        x.reshape(B, NJT, 128, 4).transpose(2, 1, 3, 0)
    ).reshape(128, 512)
    ws_np = weight_scale.reshape(1, 1)

    wb8 = packed_weight.astype(np.uint8)          # lossless: values <= 170
    in_maps = []
    for c in range(NCORES):
        sl = slice(c * OS, (c + 1) * OS)
        wbc = np.ascontiguousarray(wb8[sl, :].T)  # [JB, OS] uint8
        bc = bias[sl]
        biasrc = np.ascontiguousarray(np.broadcast_to(bc[None, :], (8, OS)))
        biascc = np.zeros((128, 1024), dtype=np.float32)
        for g in range(NG):
            for cc in range(2):
                m = 2 * g + cc
                biascc[32 * g:32 * g + 8, cc * 512:cc * 512 + CH] = (
                    bc[m * CH:(m + 1) * CH][None, :]
                )
        in_maps.append({"wb": wbc, "xs": xs_np, "biasr": biasrc,
                        "biasc": biascc, "ws": ws_np})
    return in_maps


def kernel(x, packed_weight, weight_scale, bias):
    global LAST_RESULTS
    repeat = int(os.environ.get("BITNET_REPEAT", "1"))
    mode = os.environ.get("BITNET_MODE", "full")
    blk = int(os.environ.get("BITNET_BLK", "4"))
    ep = os.environ.get("BITNET_EP", "dve")
    key = (repeat, mode, blk, ep)
    if key not in _cache:
        _cache[key] = _build(repeat, mode, blk, ep=ep)
    nc = _cache[key]

    in_maps = prepare_in_maps(
        {"x": x, "packed_weight": packed_weight,
         "weight_scale": weight_scale, "bias": bias}
    )
    res = run_bass_kernel_spmd(nc, in_maps, list(range(NCORES)))
    LAST_RESULTS = res
    return np.concatenate(
        [np.asarray(res.results[c]["out"]) for c in range(NCORES)], axis=1
    ).reshape(B, O)
